# revision 38
# baseline (speedup 1.0000x reference)
"""Karras optimal denoiser on 8 Trainium2 NeuronCores.

Math: D(x, sigma) = softmax_i(L_bi) @ y  with  L_bi = (x_b . y_i - 0.5||y_i||^2) / sigma_b^2
(the per-row constant -0.5||x||^2/sigma^2 cancels in softmax).

Sharding: train_data split over N across 8 cores. Each core returns
(num, den, mx) = (sum_i w y_i, sum_i w, rowmax L*invsig2) in its local max
frame; host does the flash-style combine.

Per-core kernel:
  GEMM1 (logits): single-pass fp16  x.y  accumulated in fp32 PSUM; the
    -0.5||y||^2 term added via a K=2 fp16 ones-matmul broadcast of the
    (hi, lo) fp16 pair of y2 (exact to ~2^-12 absolute). Chunk sizes ramp
    128/256/384 then 512 so PE work starts as soon as the first small y DMA
    lands and never outruns the stream.
  Softmax: per-chunk DVE rowmax + ACT exp (scale=1/sigma^2, bias=-chunkmax/sigma^2,
    accum_out=rowsum); after all chunks, per-chunk correction factors
    exp((cm_c - gm)/sigma^2) rescale W and the partial sums. The per-m tail is
    emitted interleaved with the last chunk so DVE work overlaps PE work.
  W^T via PE transpose-mode (fp16), then GEMM2 (num = W @ y) with W^T stationary,
    looped ki-outer over d-sections (1024,1024,512,512 wide; <=4 PSUM banks
    live), streaming small fp16 y tiles whose prefetch overlaps phase 1.
  All tile pools live for the whole kernel (no phase-boundary pool closes).
"""

import numpy as np
from contextlib import ExitStack

import concourse.bass as bass
import concourse.tile as tile
import concourse.mybir as mybir
from concourse import bacc
from concourse.bass_utils import run_bass_kernel_spmd
from concourse.masks import make_identity

dt = mybir.dt

B, C, H, W_IMG = 256, 3, 32, 32
D = C * H * W_IMG            # 3072
N_TOTAL = 50000
N_CORES = 8
NS = N_TOTAL // N_CORES      # 6250 per core
KT = D // 128                # 24 contraction k-tiles
KH = KT // 2                 # 12 k-tiles per stream half
M_TILES = 2                  # 256 query rows = 2 partition tiles
SECS = (1024, 1024, 1024)     # GEMM2 d-section widths (sum = D)


def chunk_list(ns):
    """Ramp 128/256/384 then 512 so the first y DMAs land fast."""
    sizes = [128, 256, 384]
    out = []
    off = 0
    for s in sizes:
        out.append((off, s))
        off += s
    while off < ns:
        out.append((off, min(512, ns - off)))
        off += 512
    return out


def build_nc(ns=NS):
    """Build + compile the per-core Bass program (parameterized by shard size)."""
    chunks = chunk_list(ns)
    nch = len(chunks)
    ki_n = (ns + 127) // 128          # ki tiles for GEMM2 (ynat zero-padded)
    ns_pad = ki_n * 128
    w_cols = ns_pad                    # W tile padded to transpose-tile boundary

    nc = bacc.Bacc("TRN2", target_bir_lowering=False, debug=False)

    # --- DRAM I/O ---
    y_d = [nc.dram_tensor(f"y_c{ci}", (2, 128, KH, csz), dt.float16, kind="ExternalInput").ap()
           for ci, (_, csz) in enumerate(chunks)]
    y2_d = nc.dram_tensor("y2", (2, ns), dt.float16, kind="ExternalInput").ap()
    xs_d = nc.dram_tensor("xs", (M_TILES, 128, KT, 128), dt.float16, kind="ExternalInput").ap()
    yn_d = nc.dram_tensor("ynat", (ki_n, 128, D), dt.float16, kind="ExternalInput").ap()
    is2_d = nc.dram_tensor("invsig2", (M_TILES, 128), dt.float32, kind="ExternalInput").ap()

    num_d = nc.dram_tensor("num", (M_TILES, 128, D), dt.float32, kind="ExternalOutput").ap()
    den_d = nc.dram_tensor("den", (M_TILES, 128, 1), dt.float32, kind="ExternalOutput").ap()
    mx_d = nc.dram_tensor("mx", (M_TILES, 128, 1), dt.float32, kind="ExternalOutput").ap()

    with tile.TileContext(nc) as tc:
        with ExitStack() as ctx:
            small = ctx.enter_context(tc.tile_pool(name="small", bufs=1))
            xpool = ctx.enter_context(tc.tile_pool(name="x", bufs=1))
            wpool = ctx.enter_context(tc.tile_pool(name="w", bufs=1))
            wtpool = ctx.enter_context(tc.tile_pool(name="wt", bufs=1))
            biasp = ctx.enter_context(tc.tile_pool(name="bias", bufs=8))
            outp = ctx.enter_context(tc.tile_pool(name="odrain", bufs=6))
            ynpool = ctx.enter_context(tc.tile_pool(name="ynstream", bufs=12))
            ypool = ctx.enter_context(tc.tile_pool(name="ystream", bufs=6))
            y2pool = ctx.enter_context(tc.tile_pool(name="y2p", bufs=3))
            g1ps = ctx.enter_context(tc.tile_pool(name="g1ps", bufs=2, space="PSUM"))
            tps = ctx.enter_context(tc.tile_pool(name="tps", bufs=2, space="PSUM"))
            g2ps = ctx.enter_context(tc.tile_pool(name="g2ps", bufs=4, space="PSUM"))

            # constants / small state
            ident = small.tile([128, 128], dt.float16, tag="ident")
            make_identity(nc, ident[:])
            ones2 = small.tile([2, 128], dt.float16, tag="ones2")
            nc.vector.memset(ones2[:], 1.0)
            is2_t = small.tile([128, M_TILES], dt.float32, tag="is2")
            nc.sync.dma_start(is2_t[:], is2_d.rearrange("m p -> p m"))
            cm_st = [small.tile([128, nch], dt.float32, tag=f"cm{m}", name=f"cm{m}") for m in range(M_TILES)]
            s_st = [small.tile([128, nch], dt.float32, tag=f"ss{m}", name=f"ss{m}") for m in range(M_TILES)]
            fac = [small.tile([128, nch], dt.float32, tag=f"fac{m}", name=f"fac{m}") for m in range(M_TILES)]
            scr = [small.tile([128, nch], dt.float32, tag=f"scr{m}", name=f"scr{m}") for m in range(M_TILES)]
            w_t = [wpool.tile([128, w_cols], dt.float16, tag=f"W{m}", name=f"W{m}") for m in range(M_TILES)]
            wt_t = [wtpool.tile([128, M_TILES * 128], dt.float16, tag=f"wt{t}", name=f"wt{t}") for t in range(ki_n)]

            xs_t = [xpool.tile([128, KT, 128], dt.float16, tag=f"xs{m}", name=f"xs{m}")
                    for m in range(M_TILES)]

            # chunk 0's y DMAs go out first so the smallest load lands earliest
            c0sz = chunks[0][1]
            y0h = [ypool.tile([128, KH, c0sz], dt.float16, tag="y", name=f"y0_{h}")
                   for h in range(2)]
            for h in range(2):
                nc.sync.dma_start(y0h[h][:], y_d[0][h])
            y0_2 = y2pool.tile([2, c0sz], dt.float16, tag="y2", name="y2_0")
            nc.sync.dma_start(y0_2[:], y2_d[:, :c0sz])
            y0_tiles = (y0h, y0_2)

            for m in range(M_TILES):
                # split per k-half so the first chunk's matmuls unblock sooner
                nc.sync.dma_start(xs_t[m][:, :KH, :], xs_d[m][:, :KH, :])
                nc.sync.dma_start(xs_t[m][:, KH:, :], xs_d[m][:, KH:, :])

            if w_cols > ns:
                for m in range(M_TILES):
                    nc.vector.memset(w_t[m][:, ns:w_cols], 0.0)

            # ---------------- GEMM1 + per-chunk softmax pieces ----------------
            def g1_softmax(ci, coff, csz, m, psv):
                cmsl = cm_st[m][:, ci:ci + 1]
                nc.vector.reduce_max(cmsl, psv, mybir.AxisListType.X)
                bias_t = biasp.tile([128, 1], dt.float32, tag="bias", name=f"b{ci}_{m}")
                nc.vector.tensor_scalar(
                    bias_t[:], cmsl, is2_t[:, m:m + 1], -1.0,
                    op0=mybir.AluOpType.mult, op1=mybir.AluOpType.mult)
                nc.scalar.activation(
                    w_t[m][:, coff:coff + csz], psv,
                    mybir.ActivationFunctionType.Exp,
                    bias=bias_t[:], scale=is2_t[:, m:m + 1],
                    accum_out=s_st[m][:, ci:ci + 1])

            def g1_group(ci, coff, csz, m, yh_t, y2_t):
                ps = g1ps.tile([128, 512], dt.float32, tag="g1ps", name=f"ps{ci}_{m}")
                psv = ps[:, :csz]
                # -0.5*||y||^2 broadcast: ones[2,128].T @ y2pair[2,csz] (fp16)
                nc.tensor.matmul(psv, ones2[:, :], y2_t[:, :], start=True, stop=False)
                for k in range(KT):
                    nc.tensor.matmul(
                        psv,
                        xs_t[m][:, k, :],
                        yh_t[k // KH][:, k % KH, :],
                        start=False,
                        stop=(k == KT - 1),
                    )
                g1_softmax(ci, coff, csz, m, psv)

            def softmax_tail(m):
                gm = small.tile([128, 1], dt.float32, tag=f"gm{m}", name=f"gm{m}")
                nc.vector.reduce_max(gm[:], cm_st[m][:, :], mybir.AxisListType.X)
                neg = small.tile([128, 1], dt.float32, tag=f"neg{m}", name=f"neg{m}")
                nc.vector.tensor_scalar(
                    neg[:], gm[:], is2_t[:, m:m + 1], -1.0,
                    op0=mybir.AluOpType.mult, op1=mybir.AluOpType.mult)
                mxs = small.tile([128, 1], dt.float32, tag=f"mxs{m}", name=f"mxs{m}")
                nc.vector.tensor_scalar_mul(mxs[:], neg[:], -1.0)
                nc.sync.dma_start(mx_d[m], mxs[:])
                nc.scalar.activation(
                    fac[m][:, :], cm_st[m][:, :],
                    mybir.ActivationFunctionType.Exp,
                    bias=neg[:], scale=is2_t[:, m:m + 1])
                dsb = small.tile([128, 1], dt.float32, tag=f"den{m}", name=f"den{m}")
                nc.vector.tensor_mul(scr[m][:, :], s_st[m][:, :], fac[m][:, :])
                nc.vector.reduce_sum(dsb[:], scr[m][:, :], mybir.AxisListType.X)
                nc.sync.dma_start(den_d[m], dsb[:])
                # rescale W on ScalarE (Copy with per-partition scale) so the
                # DVE queue stays free for the transpose drain copies
                # (GpSimd measured 6.5us/op here — 16x slower; DVE blocks copies)
                for ci, (coff, csz) in enumerate(chunks):
                    nc.scalar.activation(
                        w_t[m][:, coff:coff + csz], w_t[m][:, coff:coff + csz],
                        mybir.ActivationFunctionType.Copy,
                        scale=fac[m][:, ci:ci + 1])
                    if ci >= nch - 2:
                        # last chunks: transpose per-tile via the DMA xbar on
                        # the ACT HWDGE ring (parallel to PE; each wt_t tile
                        # keeps its own fine-grained dependency). Pad cols of
                        # the final tile are zeros, unrescaled is fine.
                        cpad = ((csz + 127) // 128) * 128
                        for t in range(coff // 128, (coff + cpad) // 128):
                            nc.scalar.dma_start(
                                wt_t[t][:, m * 128:(m + 1) * 128],
                                w_t[m][:, t * 128:(t + 1) * 128],
                                transpose=True)

            for ci, (coff, csz) in enumerate(chunks):
                if ci == 0:
                    yh_t, y2_t = y0_tiles     # DMA already issued before xs
                else:
                    yh_t = [ypool.tile([128, KH, csz], dt.float16, tag="y", name=f"y{ci}_{h}")
                            for h in range(2)]
                    for h in range(2):
                        nc.sync.dma_start(yh_t[h][:], y_d[ci][h])
                    y2_t = y2pool.tile([2, csz], dt.float16, tag="y2", name=f"y2_{ci}")
                    nc.sync.dma_start(y2_t[:], y2_d[:, coff:coff + csz])

                if ci < nch - 1:
                    for m in range(M_TILES):
                        g1_group(ci, coff, csz, m, yh_t, y2_t)
                else:
                    # interleave the per-m softmax tails with the last chunk's
                    # matmul groups so the DVE chain overlaps PE work
                    g1_group(ci, coff, csz, 0, yh_t, y2_t)
                    softmax_tail(0)
                    g1_group(ci, coff, csz, 1, yh_t, y2_t)
                    softmax_tail(1)

            # ------------- transposes (W[b,i] -> WT[i,b]) + GEMM2 -------------
            # keepalive transposes: no deps, keep PE busy (no HAM re-throttle)
            # while the first W rescale lands
            for wi in range(4):
                dummy = tps.tile([128, 128], dt.float16, tag="tp", name=f"warm{wi}")
                nc.tensor.matmul(dummy[:], ident[:, :], ident[:, :],
                                 is_transpose=True, start=True, stop=True)
            pe_tiles = chunks[-2][0] // 128       # last 2 chunks go via DMA xbar
            for m in range(M_TILES):              # m-outer: m=0 unblocks early
                for t in range(pe_tiles):
                    tp = tps.tile([128, 128], dt.float16, tag="tp", name=f"tp{m}_{t}")
                    nc.tensor.matmul(tp[:], w_t[m][:, t * 128:(t + 1) * 128],
                                     ident[:, :], is_transpose=True,
                                     start=True, stop=True)
                    nc.vector.tensor_copy(wt_t[t][:, m * 128:(m + 1) * 128], tp[:])

            # GEMM2: num = W @ y, ki-outer over d-sections
            doff = 0
            for sec, dsec in enumerate(SECS):
                dsub = dsec // 512
                ps = [g2ps.tile([128, 512], dt.float32, tag="g2ps", name=f"g2ps{sec}_{q}")
                      for q in range(M_TILES * dsub)]
                for ki in range(ki_n):
                    yn_t = ynpool.tile([128, dsec], dt.float16, tag="yn", name=f"yn{sec}_{ki}")
                    nc.sync.dma_start(yn_t[:], yn_d[ki][:, doff:doff + dsec])
                    for m in range(M_TILES):
                        for j in range(dsub):
                            nc.tensor.matmul(
                                ps[m * dsub + j][:],
                                wt_t[ki][:, m * 128:(m + 1) * 128],
                                yn_t[:, j * 512:(j + 1) * 512],
                                start=(ki == 0), stop=(ki == ki_n - 1))
                for m in range(M_TILES):
                    for j in range(dsub):
                        o = outp.tile([128, 512], dt.float32, tag="odrain",
                                      name=f"o{sec}_{m}_{j}")
                        # split drains across DVE and ScalarE so they run in parallel
                        if m == 0:
                            nc.vector.tensor_copy(o[:], ps[m * dsub + j][:])
                        else:
                            nc.scalar.activation(o[:], ps[m * dsub + j][:],
                                                 mybir.ActivationFunctionType.Copy)
                        nc.sync.dma_start(
                            num_d[m][:, doff + j * 512:doff + (j + 1) * 512], o[:])
                doff += dsec

    nc.compile()
    return nc


def prep_inputs(input, sigma, train_data, n_cores=N_CORES):
    """Host-side shard + pre-tile. Returns list of per-core in_maps."""
    x = np.asarray(input, dtype=np.float32).reshape(B, D)
    sig = np.asarray(sigma, dtype=np.float64)
    y = np.asarray(train_data, dtype=np.float32).reshape(N_TOTAL, D)

    x16 = x.astype(np.float16)
    # xs[m, p(d), k, b]
    xs = np.empty((M_TILES, 128, KT, 128), dtype=np.float16)
    for m in range(M_TILES):
        xm = x16[m * 128:(m + 1) * 128]          # [128b, D]
        xs[m] = xm.reshape(128, KT, 128).transpose(2, 1, 0)
    is2 = (1.0 / sig ** 2).astype(np.float32).reshape(M_TILES, 128)

    ns = N_TOTAL // n_cores
    chunks = chunk_list(ns)
    ki_n = (ns + 127) // 128
    ns_pad = ki_n * 128

    in_maps = []
    for c in range(n_cores):
        ys = y[c * ns:(c + 1) * ns]
        ys16 = ys.astype(np.float16)
        y2f = (-0.5 * np.einsum("ij,ij->i", ys.astype(np.float64), ys.astype(np.float64))
               ).astype(np.float32)
        y2h = y2f.astype(np.float16)
        y2l = (y2f - y2h.astype(np.float32)).astype(np.float16)
        y2 = np.stack([y2h, y2l])                # (2, ns)
        yn2 = np.zeros((ki_n, 128, D), dtype=np.float16)
        yn2.reshape(ns_pad, D)[:ns] = ys16
        im = {"xs": xs, "invsig2": is2, "y2": y2, "ynat": yn2}
        for ci, (coff, csz) in enumerate(chunks):
            # y_c[h, p(d), kk, s]: half h covers k-tiles h*KH..h*KH+KH-1
            yt = ys16[coff:coff + csz].T.reshape(2, KH, 128, csz)
            im[f"y_c{ci}"] = np.ascontiguousarray(yt.transpose(0, 2, 1, 3))
        in_maps.append(im)
    return in_maps


def combine(results):
    """Flash-style combine of per-core (num, den, mx) partials -> full output."""
    num = np.stack([r["num"].reshape(B, D) for r in results]).astype(np.float64)
    den = np.stack([r["den"].reshape(B) for r in results]).astype(np.float64)
    mx = np.stack([r["mx"].reshape(B) for r in results]).astype(np.float64)
    M = mx.max(axis=0)
    r = np.exp(mx - M[None, :])
    num_tot = (num * r[:, :, None]).sum(axis=0)
    den_tot = (den * r).sum(axis=0)
    out = (num_tot / den_tot[:, None]).astype(np.float32)
    return out.reshape(B, C, H, W_IMG)


_NC_CACHE = {}


def get_nc(ns=NS):
    if ns not in _NC_CACHE:
        _NC_CACHE[ns] = build_nc(ns)
    return _NC_CACHE[ns]


def kernel(input, sigma, train_data):
    nc = get_nc()
    in_maps = prep_inputs(input, sigma, train_data)
    res = run_bass_kernel_spmd(nc, in_maps, core_ids=list(range(N_CORES)))
    return combine(res.results)


# revision 40
# speedup vs baseline: 1.0725x; 1.0725x over previous
"""Karras optimal denoiser on 8 Trainium2 NeuronCores.

Math: D(x, sigma) = softmax_i(L_bi) @ y  with  L_bi = (x_b . y_i - 0.5||y_i||^2) / sigma_b^2
(the per-row constant -0.5||x||^2/sigma^2 cancels in softmax).

Sharding: train_data split over N across 8 cores. Each core returns
(num, den, mx) = (sum_i w y_i, sum_i w, rowmax L*invsig2) in its local max
frame; host does the flash-style combine.

Per-core kernel:
  GEMM1 (logits): single-pass fp16  x.y  accumulated in fp32 PSUM; the
    -0.5||y||^2 term added via a K=2 fp16 ones-matmul broadcast of the
    (hi, lo) fp16 pair of y2 (exact to ~2^-12 absolute). Chunk sizes ramp
    128/256/384 then 512 so PE work starts as soon as the first small y DMA
    lands and never outruns the stream.
  Softmax: per-chunk DVE rowmax + ACT exp (scale=1/sigma^2, bias=-chunkmax/sigma^2,
    accum_out=rowsum); after all chunks, per-chunk correction factors
    exp((cm_c - gm)/sigma^2) rescale W and the partial sums. The per-m tail is
    emitted interleaved with the last chunk so DVE work overlaps PE work.
  W^T via PE transpose-mode (fp16), then GEMM2 (num = W @ y) with W^T stationary,
    looped ki-outer over d-sections (1024,1024,512,512 wide; <=4 PSUM banks
    live), streaming small fp16 y tiles whose prefetch overlaps phase 1.
  All tile pools live for the whole kernel (no phase-boundary pool closes).
"""

import numpy as np
from contextlib import ExitStack

import concourse.bass as bass
import concourse.tile as tile
import concourse.mybir as mybir
from concourse import bacc
from concourse.bass_utils import run_bass_kernel_spmd
from concourse.masks import make_identity

dt = mybir.dt

B, C, H, W_IMG = 256, 3, 32, 32
D = C * H * W_IMG            # 3072
N_TOTAL = 50000
N_CORES = 8
NS = N_TOTAL // N_CORES      # 6250 per core
KT = D // 128                # 24 contraction k-tiles
KH = KT // 2                 # 12 k-tiles per stream half
M_TILES = 2                  # 256 query rows = 2 partition tiles
SECS = (1024, 1024, 1024)     # GEMM2 d-section widths (sum = D)


def chunk_list(ns):
    """Ramp 128/256/384 then 512 so the first y DMAs land fast."""
    sizes = [128, 256, 384]
    out = []
    off = 0
    for s in sizes:
        out.append((off, s))
        off += s
    while off < ns:
        out.append((off, min(512, ns - off)))
        off += 512
    return out


def build_nc(ns=NS):
    """Build + compile the per-core Bass program (parameterized by shard size)."""
    chunks = chunk_list(ns)
    nch = len(chunks)
    ki_n = (ns + 127) // 128          # ki tiles for GEMM2 (ynat zero-padded)
    ns_pad = ki_n * 128
    w_cols = ns_pad                    # W tile padded to transpose-tile boundary

    nc = bacc.Bacc("TRN2", target_bir_lowering=False, debug=False)

    # --- DRAM I/O ---
    y_d = [nc.dram_tensor(f"y_c{ci}", (2, 128, KH, csz), dt.float16, kind="ExternalInput").ap()
           for ci, (_, csz) in enumerate(chunks)]
    y2_d = nc.dram_tensor("y2", (2, ns), dt.float16, kind="ExternalInput").ap()
    xs_d = nc.dram_tensor("xs", (M_TILES, 128, KT, 128), dt.float16, kind="ExternalInput").ap()
    yn_d = nc.dram_tensor("ynat", (ki_n, 128, D), dt.float16, kind="ExternalInput").ap()
    is2_d = nc.dram_tensor("invsig2", (M_TILES, 128), dt.float32, kind="ExternalInput").ap()

    num_d = nc.dram_tensor("num", (M_TILES, 128, D), dt.float32, kind="ExternalOutput").ap()
    den_d = nc.dram_tensor("den", (M_TILES, 128, 1), dt.float32, kind="ExternalOutput").ap()
    mx_d = nc.dram_tensor("mx", (M_TILES, 128, 1), dt.float32, kind="ExternalOutput").ap()

    with tile.TileContext(nc) as tc:
        with ExitStack() as ctx:
            small = ctx.enter_context(tc.tile_pool(name="small", bufs=1))
            xpool = ctx.enter_context(tc.tile_pool(name="x", bufs=1))
            wpool = ctx.enter_context(tc.tile_pool(name="w", bufs=1))
            wtpool = ctx.enter_context(tc.tile_pool(name="wt", bufs=1))
            biasp = ctx.enter_context(tc.tile_pool(name="bias", bufs=8))
            outp = ctx.enter_context(tc.tile_pool(name="odrain", bufs=6))
            ynpool = ctx.enter_context(tc.tile_pool(name="ynstream", bufs=12))
            ypool = ctx.enter_context(tc.tile_pool(name="ystream", bufs=6))
            y2pool = ctx.enter_context(tc.tile_pool(name="y2p", bufs=3))
            g1ps = ctx.enter_context(tc.tile_pool(name="g1ps", bufs=2, space="PSUM"))
            tps = ctx.enter_context(tc.tile_pool(name="tps", bufs=2, space="PSUM"))
            g2ps = ctx.enter_context(tc.tile_pool(name="g2ps", bufs=4, space="PSUM"))

            # constants / small state
            ident = small.tile([128, 128], dt.float16, tag="ident")
            make_identity(nc, ident[:])
            ones2 = small.tile([2, 128], dt.float16, tag="ones2")
            nc.vector.memset(ones2[:], 1.0)
            is2_t = small.tile([128, M_TILES], dt.float32, tag="is2")
            nc.sync.dma_start(is2_t[:], is2_d.rearrange("m p -> p m"))
            cm_st = [small.tile([128, nch], dt.float32, tag=f"cm{m}", name=f"cm{m}") for m in range(M_TILES)]
            s_st = [small.tile([128, nch], dt.float32, tag=f"ss{m}", name=f"ss{m}") for m in range(M_TILES)]
            fac = [small.tile([128, nch], dt.float32, tag=f"fac{m}", name=f"fac{m}") for m in range(M_TILES)]
            scr = [small.tile([128, nch], dt.float32, tag=f"scr{m}", name=f"scr{m}") for m in range(M_TILES)]
            w_t = [wpool.tile([128, w_cols], dt.float16, tag=f"W{m}", name=f"W{m}") for m in range(M_TILES)]
            wt_t = [wtpool.tile([128, M_TILES * 128], dt.float16, tag=f"wt{t}", name=f"wt{t}") for t in range(ki_n)]

            xs_t = [xpool.tile([128, KT, 128], dt.float16, tag=f"xs{m}", name=f"xs{m}")
                    for m in range(M_TILES)]

            # chunk 0's y DMAs go out first so the smallest load lands earliest
            c0sz = chunks[0][1]
            y0h = [ypool.tile([128, KH, c0sz], dt.float16, tag="y", name=f"y0_{h}")
                   for h in range(2)]
            for h in range(2):
                nc.sync.dma_start(y0h[h][:], y_d[0][h])
            y0_2 = y2pool.tile([2, c0sz], dt.float16, tag="y2", name="y2_0")
            nc.sync.dma_start(y0_2[:], y2_d[:, :c0sz])
            y0_tiles = (y0h, y0_2)

            for m in range(M_TILES):
                # split per k-half so the first chunk's matmuls unblock sooner
                nc.sync.dma_start(xs_t[m][:, :KH, :], xs_d[m][:, :KH, :])
                nc.sync.dma_start(xs_t[m][:, KH:, :], xs_d[m][:, KH:, :])

            if w_cols > ns:
                for m in range(M_TILES):
                    nc.vector.memset(w_t[m][:, ns:w_cols], 0.0)

            # ---------------- GEMM1 + per-chunk softmax pieces ----------------
            def g1_softmax(ci, coff, csz, m, psv):
                cmsl = cm_st[m][:, ci:ci + 1]
                nc.vector.reduce_max(cmsl, psv, mybir.AxisListType.X)
                bias_t = biasp.tile([128, 1], dt.float32, tag="bias", name=f"b{ci}_{m}")
                nc.vector.tensor_scalar(
                    bias_t[:], cmsl, is2_t[:, m:m + 1], -1.0,
                    op0=mybir.AluOpType.mult, op1=mybir.AluOpType.mult)
                nc.scalar.activation(
                    w_t[m][:, coff:coff + csz], psv,
                    mybir.ActivationFunctionType.Exp,
                    bias=bias_t[:], scale=is2_t[:, m:m + 1],
                    accum_out=s_st[m][:, ci:ci + 1])

            def g1_group(ci, coff, csz, m, yh_t, y2_t):
                ps = g1ps.tile([128, 512], dt.float32, tag="g1ps", name=f"ps{ci}_{m}")
                psv = ps[:, :csz]
                # -0.5*||y||^2 broadcast: ones[2,128].T @ y2pair[2,csz] (fp16)
                nc.tensor.matmul(psv, ones2[:, :], y2_t[:, :], start=True, stop=False)
                for k in range(KT):
                    nc.tensor.matmul(
                        psv,
                        xs_t[m][:, k, :],
                        yh_t[k // KH][:, k % KH, :],
                        start=False,
                        stop=(k == KT - 1),
                    )
                g1_softmax(ci, coff, csz, m, psv)

            def softmax_tail(m):
                gm = small.tile([128, 1], dt.float32, tag=f"gm{m}", name=f"gm{m}")
                nc.vector.reduce_max(gm[:], cm_st[m][:, :], mybir.AxisListType.X)
                neg = small.tile([128, 1], dt.float32, tag=f"neg{m}", name=f"neg{m}")
                nc.vector.tensor_scalar(
                    neg[:], gm[:], is2_t[:, m:m + 1], -1.0,
                    op0=mybir.AluOpType.mult, op1=mybir.AluOpType.mult)
                mxs = small.tile([128, 1], dt.float32, tag=f"mxs{m}", name=f"mxs{m}")
                nc.vector.tensor_scalar_mul(mxs[:], neg[:], -1.0)
                nc.sync.dma_start(mx_d[m], mxs[:])
                nc.scalar.activation(
                    fac[m][:, :], cm_st[m][:, :],
                    mybir.ActivationFunctionType.Exp,
                    bias=neg[:], scale=is2_t[:, m:m + 1])
                dsb = small.tile([128, 1], dt.float32, tag=f"den{m}", name=f"den{m}")
                nc.vector.tensor_mul(scr[m][:, :], s_st[m][:, :], fac[m][:, :])
                nc.vector.reduce_sum(dsb[:], scr[m][:, :], mybir.AxisListType.X)
                nc.sync.dma_start(den_d[m], dsb[:])
                # rescale W on ScalarE (Copy with per-partition scale) so the
                # DVE queue stays free for the transpose drain copies
                # (GpSimd measured 6.5us/op here — 16x slower; DVE blocks copies)
                for ci, (coff, csz) in enumerate(chunks):
                    nc.scalar.activation(
                        w_t[m][:, coff:coff + csz], w_t[m][:, coff:coff + csz],
                        mybir.ActivationFunctionType.Copy,
                        scale=fac[m][:, ci:ci + 1])

            for ci, (coff, csz) in enumerate(chunks):
                if ci == 0:
                    yh_t, y2_t = y0_tiles     # DMA already issued before xs
                else:
                    yh_t = [ypool.tile([128, KH, csz], dt.float16, tag="y", name=f"y{ci}_{h}")
                            for h in range(2)]
                    for h in range(2):
                        nc.sync.dma_start(yh_t[h][:], y_d[ci][h])
                    y2_t = y2pool.tile([2, csz], dt.float16, tag="y2", name=f"y2_{ci}")
                    nc.sync.dma_start(y2_t[:], y2_d[:, coff:coff + csz])

                if ci < nch - 1:
                    for m in range(M_TILES):
                        g1_group(ci, coff, csz, m, yh_t, y2_t)
                else:
                    # interleave the per-m softmax tails with the last chunk's
                    # matmul groups so the DVE chain overlaps PE work
                    g1_group(ci, coff, csz, 0, yh_t, y2_t)
                    softmax_tail(0)
                    g1_group(ci, coff, csz, 1, yh_t, y2_t)
                    softmax_tail(1)

            # ------------- transposes (W[b,i] -> WT[i,b]) + GEMM2 -------------
            # keepalive transposes: no deps, keep PE busy through the boundary
            # (no HAM re-throttle) while the first W rescale lands
            # (v15 trace: boundary gap 3.3us -> 0.6us with these)
            for wi in range(4):
                dummy = tps.tile([128, 128], dt.float16, tag="tp", name=f"warm{wi}")
                nc.tensor.matmul(dummy[:], ident[:, :], ident[:, :],
                                 is_transpose=True, start=True, stop=True)
            for m in range(M_TILES):              # m-outer: m=0 unblocks early
                for t in range(ki_n):
                    tp = tps.tile([128, 128], dt.float16, tag="tp", name=f"tp{m}_{t}")
                    nc.tensor.matmul(tp[:], w_t[m][:, t * 128:(t + 1) * 128],
                                     ident[:, :], is_transpose=True,
                                     start=True, stop=True)
                    nc.vector.tensor_copy(wt_t[t][:, m * 128:(m + 1) * 128], tp[:])

            # GEMM2: num = W @ y, ki-outer over d-sections
            doff = 0
            for sec, dsec in enumerate(SECS):
                dsub = dsec // 512
                ps = [g2ps.tile([128, 512], dt.float32, tag="g2ps", name=f"g2ps{sec}_{q}")
                      for q in range(M_TILES * dsub)]
                for ki in range(ki_n):
                    yn_t = ynpool.tile([128, dsec], dt.float16, tag="yn", name=f"yn{sec}_{ki}")
                    nc.sync.dma_start(yn_t[:], yn_d[ki][:, doff:doff + dsec])
                    for m in range(M_TILES):
                        for j in range(dsub):
                            nc.tensor.matmul(
                                ps[m * dsub + j][:],
                                wt_t[ki][:, m * 128:(m + 1) * 128],
                                yn_t[:, j * 512:(j + 1) * 512],
                                start=(ki == 0), stop=(ki == ki_n - 1))
                for m in range(M_TILES):
                    for j in range(dsub):
                        o = outp.tile([128, 512], dt.float32, tag="odrain",
                                      name=f"o{sec}_{m}_{j}")
                        # split drains across DVE and ScalarE so they run in parallel
                        if m == 0:
                            nc.vector.tensor_copy(o[:], ps[m * dsub + j][:])
                        else:
                            nc.scalar.activation(o[:], ps[m * dsub + j][:],
                                                 mybir.ActivationFunctionType.Copy)
                        nc.sync.dma_start(
                            num_d[m][:, doff + j * 512:doff + (j + 1) * 512], o[:])
                doff += dsec

    nc.compile()
    return nc


def prep_inputs(input, sigma, train_data, n_cores=N_CORES):
    """Host-side shard + pre-tile. Returns list of per-core in_maps."""
    x = np.asarray(input, dtype=np.float32).reshape(B, D)
    sig = np.asarray(sigma, dtype=np.float64)
    y = np.asarray(train_data, dtype=np.float32).reshape(N_TOTAL, D)

    x16 = x.astype(np.float16)
    # xs[m, p(d), k, b]
    xs = np.empty((M_TILES, 128, KT, 128), dtype=np.float16)
    for m in range(M_TILES):
        xm = x16[m * 128:(m + 1) * 128]          # [128b, D]
        xs[m] = xm.reshape(128, KT, 128).transpose(2, 1, 0)
    is2 = (1.0 / sig ** 2).astype(np.float32).reshape(M_TILES, 128)

    ns = N_TOTAL // n_cores
    chunks = chunk_list(ns)
    ki_n = (ns + 127) // 128
    ns_pad = ki_n * 128

    in_maps = []
    for c in range(n_cores):
        ys = y[c * ns:(c + 1) * ns]
        ys16 = ys.astype(np.float16)
        y2f = (-0.5 * np.einsum("ij,ij->i", ys.astype(np.float64), ys.astype(np.float64))
               ).astype(np.float32)
        y2h = y2f.astype(np.float16)
        y2l = (y2f - y2h.astype(np.float32)).astype(np.float16)
        y2 = np.stack([y2h, y2l])                # (2, ns)
        yn2 = np.zeros((ki_n, 128, D), dtype=np.float16)
        yn2.reshape(ns_pad, D)[:ns] = ys16
        im = {"xs": xs, "invsig2": is2, "y2": y2, "ynat": yn2}
        for ci, (coff, csz) in enumerate(chunks):
            # y_c[h, p(d), kk, s]: half h covers k-tiles h*KH..h*KH+KH-1
            yt = ys16[coff:coff + csz].T.reshape(2, KH, 128, csz)
            im[f"y_c{ci}"] = np.ascontiguousarray(yt.transpose(0, 2, 1, 3))
        in_maps.append(im)
    return in_maps


def combine(results):
    """Flash-style combine of per-core (num, den, mx) partials -> full output."""
    num = np.stack([r["num"].reshape(B, D) for r in results]).astype(np.float64)
    den = np.stack([r["den"].reshape(B) for r in results]).astype(np.float64)
    mx = np.stack([r["mx"].reshape(B) for r in results]).astype(np.float64)
    M = mx.max(axis=0)
    r = np.exp(mx - M[None, :])
    num_tot = (num * r[:, :, None]).sum(axis=0)
    den_tot = (den * r).sum(axis=0)
    out = (num_tot / den_tot[:, None]).astype(np.float32)
    return out.reshape(B, C, H, W_IMG)


_NC_CACHE = {}


def get_nc(ns=NS):
    if ns not in _NC_CACHE:
        _NC_CACHE[ns] = build_nc(ns)
    return _NC_CACHE[ns]


def kernel(input, sigma, train_data):
    nc = get_nc()
    in_maps = prep_inputs(input, sigma, train_data)
    res = run_bass_kernel_spmd(nc, in_maps, core_ids=list(range(N_CORES)))
    return combine(res.results)
